# revision 52
# baseline (speedup 1.0000x reference)
"""Trainium2 Bass kernel for nn_DendriticLayerSiLU_Template.

out = silu(g) * (x @ W.T), where per (token n, unit h):
  a[n,h,w] = sum_s x[n, w*64+s] * T[h, w*64+s]      (32 windows of size 64)
  p = softmax(|a| / tau), tau=1  (over w)
  g[n,h] = sum_w p[n,h,w] * a[n,h,w]

Strategy: 8-way data-parallel over N=4096 tokens (512/core), fp16 on-chip.
The gate's elementwise pipeline is the wall (ACT: PSUM drain + exp; DVE:
abs + a*e + reduction trees), so the kernel is software-pipelined at
half-unit (16-window) granularity with one slot of lookahead:

  slot k:  PE   einsum half k (16 windows, pair matmuls -> PSUM pairs)
           ACT  drain pairs of half k, exp(|a|) of half k (2x8 windows)
           DVE  mult0+mult1 of half k-1, then abs/exps of half k woven
                through half k-1's plane-fused reduction tree

The two 8-window mults stay split: mult0 depends only on the previous
slot's first exp (lands mid-slot, sem latency hidden); a fused 16-window
mult would wait on the last exp at the slot boundary and stall DVE every
slot. The reduction trees fuse the num/den planes into single ops
(3 levels + merge). Slot 0 streams at 4-window granularity to pull the
DVE chain forward. Tails (merge halves, g = num/den, silu(g)*lin,
store) are batched per token tile — both h-chunks in one set of
double-size ops every 4th slot — halving tail op count and sem traffic;
the final store is split so the span-ending DMA overlaps its compute.
lin = x@W.T runs as one PE burst per token-tile inside slots 2-5 (PE is
~35% busy) using a dedicated 2-bank PSUM tile; DMA streams inputs as
(xT_c, tT_c) pairs then wT so the first einsum starts within ~2us.
"""

import sys

if "/opt/trn_rl_repo" not in sys.path:
    sys.path.insert(0, "/opt/trn_rl_repo")

import numpy as np

import concourse.bass as bass
import concourse.tile as tile
from concourse import bacc, mybir
from concourse.bass_utils import run_bass_kernel_spmd

# Problem shapes (hardcoded per harness contract)
N_TOKENS = 4096
D = 2048          # in_features
H = 1024          # out_features
WIN = 64          # window size
NW = 32           # num windows
NCORES = 8
TOK = N_TOKENS // NCORES    # tokens per core = 512
NTT = TOK // 128            # token tiles per core = 4
NHC = H // 512              # h chunks = 2
KC = D // 128               # k chunks for linear = 16

F16 = mybir.dt.float16
F32 = mybir.dt.float32
U16 = mybir.dt.uint16


def _build_module():
    nc = bacc.Bacc("TRN2", target_bir_lowering=False, debug=False,
                   num_devices=NCORES)

    xT = nc.dram_tensor("xT", [D, TOK], F16, kind="ExternalInput").ap()
    wT = nc.dram_tensor("wT", [D, H], F16, kind="ExternalInput").ap()
    tT = nc.dram_tensor("tT", [D, H], F16, kind="ExternalInput").ap()
    out = nc.dram_tensor("out", [TOK, H], F32, kind="ExternalOutput").ap()

    with tile.TileContext(nc) as tc, nc.allow_low_precision(
        reason="fp16 gate pipeline by design"
    ):
        _body(tc, nc, xT, wT, tT, out)

    nc.compile()
    return nc


class _HalfSlot:
    """One 16-window half-unit: the nd tile plus unit bookkeeping."""

    def __init__(self, nd, ndh, half, tt, hc):
        self.nd = nd       # [128, 2(a|e), 16, 512] fp16
        self.ndh = ndh     # per-unit [128, 2(half), 2(num|den), 512]
        self.half = half
        self.tt = tt
        self.hc = hc


def _body(tc, nc, xT, wT, tT, out):
    from contextlib import ExitStack

    from concourse.dve_ops import (
        RECIPROCAL_APPROX_FAST, RECIP_APPROX_FAST_CONSTS)

    ctx = ExitStack()
    with ctx:
        weights = ctx.enter_context(tc.tile_pool(name="weights", bufs=1))
        nd_p = ctx.enter_context(tc.tile_pool(name="nd", bufs=3))
        ndh_p = ctx.enter_context(tc.tile_pool(name="ndh", bufs=2))
        # tails are 4 slots apart; their scratch never overlaps
        smalls = ctx.enter_context(tc.tile_pool(name="smalls", bufs=1))
        # tails are 4 slots (~60us) apart — the previous out-DMA is long
        # gone before the next tail writes, so one staging buffer suffices
        outs_p = ctx.enter_context(tc.tile_pool(name="outs", bufs=1))
        ppool = ctx.enter_context(tc.tile_pool(name="apsum", bufs=3,
                                               space="PSUM"))
        lpool = ctx.enter_context(tc.tile_pool(name="lpsum", bufs=1,
                                               space="PSUM"))

        # ---- resident inputs (fp16, pre-transposed on host) ----
        # DMA in (xT_c, tT_c, wT_c) triplets: the slot-0 einsum needs only
        # (xT_c, tT_c) pairs first — the einsum slots need only those, so
        # the PE/drain ramp is paced at ~1.1us per chunk — then wT, which
        # is first needed by the lin burst in slot 2 (~35us in).
        xT_t, wT_t, tT_t = [], [], []
        for c in range(KC):
            xt = weights.tile([128, TOK], F16, name=f"xT{c}", tag=f"xT{c}")
            nc.sync.dma_start(out=xt[:], in_=xT[c * 128:(c + 1) * 128, :])
            xT_t.append(xt)
            tt_ = weights.tile([128, H], F16, name=f"tT{c}", tag=f"tT{c}")
            nc.sync.dma_start(out=tt_[:], in_=tT[c * 128:(c + 1) * 128, :])
            tT_t.append(tt_)
        for c in range(KC):
            wt = weights.tile([128, H], F16, name=f"wT{c}", tag=f"wT{c}")
            nc.sync.dma_start(out=wt[:], in_=wT[c * 128:(c + 1) * 128, :])
            wT_t.append(wt)

        lin_bf = [weights.tile([128, 2, 512], F16, name=f"lin{t}",
                               tag=f"lin{t}") for t in range(NTT)]

        # ---- helpers -----------------------------------------------------
        def emit_pe_drains(slot, prs=range(8)):
            """PE einsum pairs + ACT pair drains for half-slot `slot`."""
            nd, tt, hc, half = slot.nd, slot.tt, slot.hc, slot.half
            tok_sl = bass.ts(tt, 128)
            h_sl = bass.ts(hc, 512)
            for pr in prs:
                aps = ppool.tile([128, 2, 512], F32, tag="aps", name="aps")
                for i in range(2):
                    w = half * 16 + pr * 2 + i
                    ct, ro = w // 2, (w % 2) * WIN
                    nc.tensor.matmul(
                        aps[:, i, :],
                        lhsT=xT_t[ct][ro:ro + WIN, tok_sl],
                        rhs=tT_t[ct][ro:ro + WIN, h_sl],
                        start=True, stop=True,
                    )
                nc.scalar.copy(out=nd[:, 0, pr * 2:pr * 2 + 2, :],
                               in_=aps[:, :, :])

        def emit_abs(slot, grp, nw=4):
            """|a| -> plane 1 for one window group (DVE int16 4x mode)."""
            gs = slice(grp * nw, grp * nw + nw)
            nd = slot.nd
            nc.vector.tensor_scalar(
                out=nd[:, 1, gs, :].bitcast(U16),
                in0=nd[:, 0, gs, :].bitcast(U16),
                scalar1=0x7FFF, scalar2=None,
                op0=mybir.AluOpType.bitwise_and,
            )

        def emit_exp(slot, q, nw=8):
            """e = exp(|a|) in place on plane 1 (8-window ACT op)."""
            qs = slice(q * nw, (q + 1) * nw)
            nd = slot.nd
            nc.scalar.activation(
                out=nd[:, 1, qs, :], in_=nd[:, 1, qs, :],
                func=mybir.ActivationFunctionType.Exp,
            )

        def emit_mult(slot, q, nw=8):
            """prod = a * e in-place on plane 0 (8-window tensor_tensor)."""
            qs = slice(q * nw, (q + 1) * nw)
            nd = slot.nd
            nc.vector.tensor_tensor(
                out=nd[:, 0, qs, :], in0=nd[:, 0, qs, :],
                in1=nd[:, 1, qs, :], op=mybir.AluOpType.mult,
            )

        def emit_tree_l1(slot):
            """First tree level, num/den planes fused: 16 windows -> 8."""
            nd = slot.nd
            nc.vector.tensor_tensor(
                out=nd[:, :, 0:8, :], in0=nd[:, :, 0:8, :],
                in1=nd[:, :, 8:16, :], op=mybir.AluOpType.add,
            )

        def emit_tree_l2(slot):
            """Second level, plane-fused: 8 -> 4."""
            nd = slot.nd
            nc.vector.tensor_tensor(
                out=nd[:, :, 0:4, :], in0=nd[:, :, 0:4, :],
                in1=nd[:, :, 4:8, :], op=mybir.AluOpType.add,
            )

        def emit_tree_l3m(slot):
            """Level 4->2 then merge -> ndh[half, hc]."""
            nd = slot.nd
            nc.vector.tensor_tensor(
                out=nd[:, :, 0:2, :], in0=nd[:, :, 0:2, :],
                in1=nd[:, :, 2:4, :], op=mybir.AluOpType.add,
            )
            nc.vector.tensor_tensor(
                out=slot.ndh[:, slot.half, slot.hc, :, :],
                in0=nd[:, :, 0, :], in1=nd[:, :, 1, :],
                op=mybir.AluOpType.add,
            )

        def emit_tree(slot):
            emit_tree_l1(slot)
            emit_tree_l2(slot)
            emit_tree_l3m(slot)

        def emit_lin(t):
            """One token tile of lin = x @ W.T: PE burst + ACT drain."""
            tok_sl = bass.ts(t, 128)
            lps = lpool.tile([128, 2, 512], F32, tag="lps", name="lps")
            for hc in range(NHC):
                for k in range(KC):
                    nc.tensor.matmul(
                        lps[:, hc, :],
                        lhsT=xT_t[k][:, tok_sl],
                        rhs=wT_t[k][:, bass.ts(hc, 512)],
                        start=(k == 0), stop=(k == KC - 1),
                    )
            # drain pre-scaled by 0.5 (free on ACT) so the tail's
            # g/2 * lin product is a plain 2x tensor_tensor
            nc.scalar.mul(out=lin_bf[t][:], in_=lps[:, :, :], mul=0.5)

        def emit_tail(tt, ndh, split_dma=False):
            """One token tile's tail, both h-chunks batched: merge halves,
            g = num/den, out = silu(g)*lin, DMA. Batching halves the tail
            op count (and its sem traffic) at double the op sizes."""
            nc.vector.tensor_tensor(
                out=ndh[:, 0], in0=ndh[:, 0],
                in1=ndh[:, 1], op=mybir.AluOpType.add)
            rcp = smalls.tile([128, 2, 512], F16, tag="rcp")
            nc.vector._custom_dve(
                RECIPROCAL_APPROX_FAST, out=rcp[:], in0=ndh[:, 0, :, 1, :],
                **RECIP_APPROX_FAST_CONSTS)
            g = ndh[:, 1, :, 0, :]  # recycle: half-1 is dead post-merge
            nc.vector.tensor_tensor(
                out=g, in0=ndh[:, 0, :, 0, :], in1=rcp[:],
                op=mybir.AluOpType.mult)
            # silu(g) = g * (1 + tanh(g/2)) / 2; tanh shares the exp ACT
            # table set -> no table switches.
            th = rcp  # recycle
            nc.scalar.activation(
                out=th[:], in_=g,
                func=mybir.ActivationFunctionType.Tanh, scale=0.5,
            )
            gl = ndh[:, 0, :, 0, :]  # recycle dead slot
            nc.vector.tensor_tensor(
                out=gl, in0=g, in1=lin_bf[tt][:, :, :],
                op=mybir.AluOpType.mult)
            o = outs_p.tile([128, 2, 512], F32, tag="o")
            if not split_dma:
                nc.vector.scalar_tensor_tensor(
                    out=o[:], in0=th[:], scalar=1.0, in1=gl,
                    op0=mybir.AluOpType.add, op1=mybir.AluOpType.mult)
                nc.sync.dma_start(out=out[bass.ts(tt, 128), :], in_=o[:])
            else:
                # final tail: store in quarters so the span-ending DMA is
                # one 128KB piece overlapped with the preceding compute
                for hh in range(4):
                    hc_i, ch = divmod(hh, 2)
                    cs = slice(ch * 256, ch * 256 + 256)
                    nc.vector.scalar_tensor_tensor(
                        out=o[:, hc_i, cs], in0=th[:, hc_i, cs],
                        scalar=1.0, in1=ndh[:, 0, hc_i, 0, cs],
                        op0=mybir.AluOpType.add, op1=mybir.AluOpType.mult)
                    nc.sync.dma_start(
                        out=out[bass.ts(tt, 128),
                                hc_i * 512 + ch * 256:
                                hc_i * 512 + ch * 256 + 256],
                        in_=o[:, hc_i, cs])

        # ---- main loop: 16 half-slots, one slot of lookahead --------------
        prev = None        # _HalfSlot whose DVE mult/tree runs this slot
        ndh_cur = None
        slot_idx = 0
        for tt in range(NTT):
            for hc in range(NHC):
                for half in range(2):
                    if hc == 0 and half == 0:
                        # per-token-tile: [half, hc, num|den, 512]
                        ndh_cur = ndh_p.tile([128, 2, 2, 2, 512], F16,
                                             tag="ndh")
                    nd_tile = nd_p.tile([128, 2, 16, 512], F16, tag="nd",
                                        name="nd")
                    cur = _HalfSlot(nd_tile, ndh_cur, half, tt, hc)
                    if slot_idx == 0:
                        # pipeline-fill: no carried DVE work exists yet, so
                        # stream slot 0 at 4-window granularity (2-window
                        # for the very first group, so the first abs/exp
                        # issue one drain earlier) — each exp starts right
                        # after its drains + abs, and the first mult lands
                        # early, pulling the whole DVE chain forward
                        emit_pe_drains(cur, range(0, 1))
                        emit_abs(cur, 0, nw=2)
                        emit_exp(cur, 0, nw=2)
                        emit_pe_drains(cur, range(1, 2))
                        emit_abs(cur, 1, nw=2)
                        emit_exp(cur, 1, nw=2)
                        for q in range(1, 4):
                            emit_pe_drains(cur, range(2 * q, 2 * q + 2))
                            emit_abs(cur, q)
                            emit_exp(cur, q, nw=4)
                            emit_mult(cur, q - 1, nw=4)
                        emit_mult(cur, 3, nw=4)
                        cur.mults_done = True
                        prev = cur
                        slot_idx += 1
                        continue
                    emit_pe_drains(cur)
                    # DVE weave, stall-free against ACT's [8 drains, e0,
                    # e1]. The mult pair is rotated across the slot
                    # boundary: mult0 of slot k runs at the END of slot k
                    # (its e0 lands mid-slot), mult1 at the start of slot
                    # k+1 (its e1 lands at the boundary) — this fills the
                    # ramp's idle and drops one mult from the flush. abs
                    # groups land as their drains finish; each exp finds
                    # ACT just freeing up.
                    if not getattr(prev, "mults_done", False):
                        emit_mult(prev, 1)
                    emit_tree_l1(prev)
                    emit_abs(cur, 0, nw=8)
                    emit_exp(cur, 0)
                    emit_tree_l2(prev)
                    emit_abs(cur, 1, nw=8)
                    emit_exp(cur, 1)
                    emit_tree_l3m(prev)
                    # lin bursts occupy PE/ACT slack in slots 2-5: late
                    # enough that wT has streamed in, and slot 2 emits
                    # lin(tt0) just before unit (tt0,hc0)'s tail uses it
                    if 2 <= slot_idx <= NTT + 1:
                        emit_lin(slot_idx - 2)
                    # a finished token tile's batched tail
                    if prev.half == 1 and prev.hc == 1:
                        emit_tail(prev.tt, prev.ndh)
                    # pre-run cur's first mult against the just-emitted e0
                    emit_mult(cur, 0)
                    prev = cur
                    slot_idx += 1

        # ---- pipeline flush (mult0 of the last slot was pre-run) ----
        emit_mult(prev, 1)
        emit_tree(prev)
        emit_tail(prev.tt, prev.ndh, split_dma=True)


_NC_CACHE = None


def _get_module():
    global _NC_CACHE
    if _NC_CACHE is None:
        _NC_CACHE = _build_module()
    return _NC_CACHE


def kernel(x: np.ndarray, template_flat: np.ndarray,
           weights: np.ndarray) -> np.ndarray:
    nc = _get_module()

    xT = np.ascontiguousarray(x.T.astype(np.float16))           # [D, N]
    wT = np.ascontiguousarray(weights.T.astype(np.float16))     # [D, H]
    tT = np.ascontiguousarray(template_flat.T.astype(np.float16))

    in_maps = []
    for c in range(NCORES):
        in_maps.append({
            "xT": np.ascontiguousarray(xT[:, c * TOK:(c + 1) * TOK]),
            "wT": wT,
            "tT": tT,
        })
    res = run_bass_kernel_spmd(nc, in_maps, core_ids=list(range(NCORES)))
    return np.concatenate([res.results[c]["out"] for c in range(NCORES)],
                          axis=0).astype(np.float32)

